# revision 16
# baseline (speedup 1.0000x reference)
"""GroupPointTransformer Trainium2 kernel (8 NeuronCores).

Strategy:
  - batch b (2) x 4-way shard of the N=131072 points -> 8 cores.
  - Host: per (b, shard) sort points by segment id, pad each 128-segment
    window to whole 128-point tiles (common schedule across cores so the
    SPMD program is input-value independent in shape). Host also folds
    weight products (fg1@fd2, fg1@k3n, fg1-premultiplied q table fqg =
    fg1@(q+c_s) gathered per point) and byte-packs the streams; all
    N-scale GEMMs/nonlinearities stay on device.
  - Device per 512-point macro (emission skewed 4 macros: front half
    [DMA, pe1, t] runs ahead of back half [a, w, ev, scatter]):
      pe1 = relu(fd1a @ [d;1])           (bias via K=4 ones-row; Act
                                          writes fp8 into pe1 tile)
      t   = relu(fd2fg1@pe1 + k3fg1@xf + ident@fqg + fg_b1)
            -- the s stage folded through fg1; xf enters as a K=3
            matmul straight from the bulk xf table; relu on DVE
      a,w = point-major data-stationary matmuls (w as two unpaired MMs:
            pe1-slice@fd2T fp8 + xf-slice@v3); e = exp(a/sqrt(128))
      segment sums of [e, e*(v+pe)] via fp8 one-hot scatter matmuls;
      adjacent same-window tiles pair into fp8 DoubleRow matmuls (2x).
    Bias/softmax folds: fg_b2 dropped (per-feature softmax invariant),
    max-subtraction dropped (logits O(0.01), shift-invariant), w bias
    cw=wv@b0+fd_b2 folded into the output bias since sum(attn)=1.
  - DMA: d4/xf ship once as compact bulk tables (the old per-macro
    xfd8 stream was ~97% zero padding); per-macro only the qgoh stream
    (bf16 fqg + fp8 one-hot) via Sync HWDGE. cc_in/tail DMAs ride Pool
    SWDGE so they never contend with the stream ring.
  - ReduceScatter (bf16) per 4-core group, pipelined chunks; the whole
    tail (res = numer/denom, out = fc2 @ res + nfo) is deferred past
    the main loop via tile_wait_until so RS-completion waits never
    head-of-line-block a stream queue mid-kernel.
"""

import math

import ml_dtypes
import numpy as np

import concourse.bacc as bacc
import concourse.bass as bass
import concourse.mybir as mybir
import concourse.tile as tile
from concourse.bass_utils import run_bass_kernel_spmd

B, N, M, DP, DM = 2, 131072, 4096, 3, 128
NCORE = 8
GROUP = 4                    # cores per batch
NS = N // GROUP              # points per core = 32768
NWIN = M // 128              # 32 windows of 128 segments
BF16 = mybir.dt.bfloat16
F32 = mybir.dt.float32
F8 = mybir.dt.float8e4
F16 = mybir.dt.float16
U8 = mybir.dt.uint8
NPBF16 = ml_dtypes.bfloat16
NPF8 = ml_dtypes.float8_e4m3
ISQ = 1.0 / math.sqrt(DM)
RG = [[0, 1, 2, 3], [4, 5, 6, 7]]
# reduce-scatter pipeline chunks (windows per chunk): big early chunks
# overlap the main loop; small late ones keep the serial CC-core chain and
# the final RS short.
W_CH = [8, 8, 8, 4, 4]
NCHUNK = len(W_CH)
CW = [0]
for _w in W_CH:
    CW.append(CW[-1] + _w)
SEGR_C = [32 * w for w in W_CH]      # per-core rows per chunk
OFF_C = [0]
for _s in SEGR_C:
    OFF_C.append(OFF_C[-1] + _s)
SEGR_TOT = OFF_C[-1]                 # = M // GROUP = 1024

# weight-pack column offsets in wpk [128, 1024] bf16
WC_FD2, WC_FG1, WC_RHSA, WC_ID = 0, 128, 256, 384
WC_K3, WC_V3, WC_FD1A, WC_FC2 = 512, 640, 768, 896


def _build(nc, tiles_w, no_cc=False):
    """Emit the SPMD program. tiles_w[w] = # of 128-point tiles in window w
    (common across all cores)."""
    T = int(sum(tiles_w))
    assert T % 4 == 0
    nmacro = T // 4
    win_of = np.repeat(np.arange(NWIN), tiles_w)
    first_t = np.zeros(NWIN, np.int64)
    last_t = np.zeros(NWIN, np.int64)
    o = 0
    for w in range(NWIN):
        first_t[w] = o
        o += tiles_w[w]
        last_t[w] = o - 1

    # ---- I/O ----
    # bulk point table: d4a = [d;1] bf16 (one upfront DMA, SBUF-resident)
    d4a_d = nc.dram_tensor("d4a", [4, T * 128], BF16, kind="ExternalInput")
    # byte-packed per-macro stream: 0:1024 qg bf16 | 1024:1536 oh fp8 |
    # 1536:2048 xf fp8 (rows 0:3; rest zeros).  The SBUF tile has a pe1
    # slot at 2048:2560 so (xf, pe1) form adjacent DoubleRow planes.
    qgoh_d = nc.dram_tensor("qgoh", [T // 4, 128, 2048], U8,
                            kind="ExternalInput")
    wpk_d = nc.dram_tensor("wpk", [128, 1024], BF16, kind="ExternalInput")
    wf8_d = nc.dram_tensor("wf8", [128, 512], F8, kind="ExternalInput")
    bt_d = nc.dram_tensor("bt", [DM, 1], F32, kind="ExternalInput")
    nfo_d = nc.dram_tensor("nfo", [DP, SEGR_TOT], F32, kind="ExternalInput")
    out_d = nc.dram_tensor("out", [DP, SEGR_TOT], F32, kind="ExternalOutput")

    cc_in = nc.dram_tensor("cc_in", [M, 256], F16)
    cc_out = [nc.dram_tensor(f"cc_out{c}", [SEGR_C[c], 256], F16)
              for c in range(NCHUNK)]

    AF = mybir.ActivationFunctionType
    AL = mybir.AluOpType

    with tile.TileContext(nc) as tc:
        with (
            tc.tile_pool(name="cpool", bufs=1) as cp,
            tc.tile_pool(name="spool", bufs=8) as sp,
            tc.tile_pool(name="psum", bufs=1, space="PSUM") as pp,
        ):
            # ---- constants ----
            wpk = cp.tile([128, 1024], BF16, tag="wpk", name="wpk")
            nc.scalar.dma_start(wpk[:], wpk_d[:])

            fd2T = wpk[:, WC_FD2:WC_FD2 + 128]
            rhsa = wpk[:, WC_RHSA:WC_RHSA + 128]
            ident = wpk[:, WC_ID:WC_ID + 128]
            fd1aT = wpk[0:4, WC_FD1A:WC_FD1A + 128]
            fc2T = wpk[:, WC_FC2:WC_FC2 + 3]
            # fp8 weights: [fd2fg1T | k3fg1T | fd2T | v3]
            # fp8 weight pairs, plane order (xf, pe1): [k3n|fd2fg1], [v3|fd2T]
            wf8 = cp.tile([128, 512], F8, tag="wf8", name="wf8")
            bt = cp.tile([DM, 1], F32, tag="bt", name="bt")
            kfd8 = wf8[:, 0:256].rearrange("p (a f) -> p a f", a=2)
            vfd8 = wf8[:, 256:512].rearrange("p (a f) -> p a f", a=2)

            d4a = cp.tile([4, T * 128], BF16, tag="d4a", name="d4a")

            # ---- main loop over 512-point macros ----
            # software-pipelined emission: front half (DMA, pe1, t) of macro
            # mi is emitted before the back half (a, w, ev, scatter) of
            # macro mi-1, so each engine's program order matches data-ready
            # order (Act/DVE have shallow lookahead queues).
            live = {}
            closed = np.zeros(NWIN, bool)
            fr = {}
            fr_sc = {}

            pf = {}

            def prefetch(mi):
                qc = sp.tile([128, 2560], U8, tag="qgoh", name="qc")
                nc.sync.dma_start(qc[:, 0:2048], qgoh_d[mi][:])
                pf[mi] = qc

            def front(mi):
                qc = pf.pop(mi)
                fqg = qc[:, 0:1024].bitcast(BF16)
                sl = slice(mi * 512, (mi + 1) * 512)
                pe1_ps = pp.tile([128, 512], F32, tag="pe1", bufs=1)
                nc.tensor.matmul(pe1_ps[:], fd1aT, d4a[:, sl],
                                 start=True, stop=True)
                nc.scalar.activation(qc[:, 2048:2560].bitcast(F8), pe1_ps[:],
                                     AF.Relu)

                # t = (fg1 fd2)@pe1 + (fg1 k3n)@xf + fg1@(q+c_s)[idx]:
                # one fp8 DoubleRow over (xf, pe1) + one ident matmul for fqg
                t_ps = pp.tile([128, 512], F32, tag="t", bufs=2)
                comb2 = qc[:, 1536:2560].bitcast(F8).rearrange(
                    "p (a f) -> p a f", a=2)
                nc.tensor.matmul(t_ps[:], kfd8, comb2, start=True, stop=False,
                                 perf_mode=mybir.MatmulPerfMode.DoubleRow)
                nc.tensor.matmul(t_ps[:], ident, fqg, start=False, stop=True)
                t_sb = sp.tile([128, 512], BF16, tag="t")
                # relu lives on DVE; Act keeps pe1-relu + exp + sc copies
                nc.vector.tensor_scalar(t_sb[:], t_ps[:], bt[:], 0.0,
                                        AL.add, AL.max)
                fr[mi] = (qc, t_sb)

            def back(mi):
                t0 = 4 * mi
                qc, t_sb = fr.pop(mi)
                oh4 = qc[:, 1024:1536].bitcast(F8).rearrange(
                    "p (a f) -> p a f", a=4)
                comb2 = qc[:, 1536:2560].bitcast(F8).rearrange(
                    "p (a f) -> p a f", a=2)

                # point-major: w = (v + pe)^T, a = (fg2 t)^T
                w_ps = pp.tile([128, 4, 128], F32, tag="w", bufs=2, name="w_ps")
                a_ps = pp.tile([128, 4, 128], F32, tag="a", bufs=1, name="a_ps")
                for k in range(4):
                    sl = slice(k * 128, (k + 1) * 128)
                    nc.tensor.matmul(a_ps[:, k, :], t_sb[:, sl], rhsa,
                                     start=True, stop=True)
                for k in range(4):
                    nc.tensor.matmul(w_ps[:, k, :],
                                     comb2[:, :, k * 128:(k + 1) * 128], vfd8,
                                     start=True, stop=True,
                                     perf_mode=mybir.MatmulPerfMode.DoubleRow)

                # e and e*w interleaved per subtile: [pts, (k, e|ew)], fp8
                ev_sb = sp.tile([128, 4, 256], F8, tag="ev")
                nc.scalar.activation(ev_sb[:, :, 0:128], a_ps[:], AF.Exp,
                                     scale=ISQ)
                nc.vector.tensor_mul(ev_sb[:, :, 128:256], ev_sb[:, :, 0:128],
                                     w_ps[:])

                # scatter into per-window PSUM accumulators; adjacent tiles of
                # the same window pair up into one fp8 DoubleRow matmul (2x)
                k = 0
                while k < 4:
                    t = t0 + k
                    w = int(win_of[t])
                    if w // 2 not in live:
                        live[w // 2] = pp.tile([128, 512], F32, tag="sc",
                                               bufs=2, name=f"sc{w // 2}")
                    paired = k < 3 and int(win_of[t + 1]) == w
                    st = t == first_t[w]
                    reg = live[w // 2][:, (w % 2) * 256:(w % 2) * 256 + 256]
                    if paired:
                        fin = t + 1 == last_t[w]
                        nc.tensor.matmul(reg, oh4[:, k:k + 2, :],
                                         ev_sb[:, k:k + 2, :], start=st, stop=fin,
                                         perf_mode=mybir.MatmulPerfMode.DoubleRow)
                        k += 2
                    else:
                        fin = t == last_t[w]
                        nc.tensor.matmul(reg, oh4[:, k, :],
                                         ev_sb[:, k, :], start=st, stop=fin)
                        k += 1
                    if fin:
                        closed[w] = True
                        # drain each window half as it closes so the shared
                        # PSUM bank frees right after the second half-copy
                        if w % 2 == 0:
                            sc2 = sp.tile([128, 2, 256], F16, tag="scsb",
                                          name="sc2")
                            fr_sc[w // 2] = sc2
                            nc.scalar.copy(sc2[:, 0, :],
                                           live[w // 2][:, 0:256])
                        else:
                            sc2 = fr_sc.pop(w // 2)
                            if (w // 2) % 2 == 0:
                                nc.scalar.copy(sc2[:, 1, :],
                                               live[w // 2][:, 256:512])
                            else:
                                nc.vector.tensor_copy(sc2[:, 1, :],
                                                      live[w // 2][:, 256:512])
                            nc.gpsimd.dma_start(
                                cc_in[(w - 1) * 128:(w + 1) * 128, :]
                                .rearrange("(a p) f -> p a f", p=128), sc2[:])
                            del live[w // 2]
                        for c in range(NCHUNK):
                            if (not no_cc and CW[c] <= w < CW[c + 1]
                                    and closed[CW[c]:CW[c + 1]].all()):
                                nc.gpsimd.collective_compute(
                                    "ReduceScatter", AL.add, replica_groups=RG,
                                    ins=[cc_in[CW[c] * 128:CW[c + 1] * 128, :]],
                                    outs=[cc_out[c][:]])

            # head loads ride the scalar HWDGE ring so the stream owns sync
            nc.scalar.dma_start(d4a[:], d4a_d[:])
            nc.scalar.dma_start(wf8[:], wf8_d[:])
            nc.scalar.dma_start(bt[:], bt_d[:])
            prefetch(0)
            prefetch(1)
            SKEW = 4
            for mi in range(nmacro + SKEW):
                if mi + 2 < nmacro:
                    prefetch(mi + 2)
                if mi < nmacro:
                    front(mi)
                if mi >= SKEW:
                    back(mi - SKEW)

            # ---- tail: res = numer/denom; out = fc2 @ res + nfo ----
            # deferred past the whole main loop (tile_wait_until) so the
            # RS-completion waits sit at the end of every engine queue and
            # never head-of-line-block the stream mid-kernel.
            with tc.tile_wait_until(10.0):
                for c in range(NCHUNK):
                    NA = SEGR_C[c] // 128
                    osl = slice(OFF_C[c], OFF_C[c + 1])
                    tt = sp.tile([128, NA, 256], F16, tag="tt", bufs=2,
                                 padded_shape=[128, 2, 256])
                    nc.gpsimd.dma_start(
                        tt[:], cc_out[c].rearrange("(a p) f -> p a f", p=128))
                    dmx = sp.tile([128, NA, 128], F32, tag="dmx", bufs=2,
                                  padded_shape=[128, 2, 128])
                    nc.vector.tensor_scalar_max(dmx[:], tt[:, :, 0:128], 1e-30)
                    rec = sp.tile([128, NA, 128], F32, tag="rec", bufs=2,
                                  padded_shape=[128, 2, 128])
                    nc.vector.reciprocal_approx_fast(rec[:], dmx[:])
                    res = sp.tile([128, NA, 128], BF16, tag="res", bufs=2,
                                  padded_shape=[128, 2, 128])
                    nc.vector.tensor_mul(res[:], tt[:, :, 128:256], rec[:])
                    rT_ps = pp.tile([128, SEGR_C[c]], BF16, tag="pe1", bufs=1,
                                    name="rT_ps", padded_shape=[128, 256])
                    for a in range(NA):
                        nc.tensor.transpose(rT_ps[:, a * 128:(a + 1) * 128],
                                            res[:, a, :], ident)
                    rT_sb = sp.tile([128, SEGR_C[c]], BF16, tag="rT", bufs=2,
                                    padded_shape=[128, 256])
                    nc.scalar.copy(rT_sb[:], rT_ps[:])
                    o_ps = pp.tile([DP, SEGR_C[c]], F32, tag="a", bufs=1,
                                   name="o_ps", padded_shape=[DP, 256])
                    nc.tensor.matmul(o_ps[:], fc2T, rT_sb[:], start=True,
                                     stop=True)
                    nfo_sb = sp.tile([DP, SEGR_C[c]], F32, tag="nfo", bufs=2,
                                     padded_shape=[DP, 256])
                    nc.gpsimd.dma_start(nfo_sb[:], nfo_d[:, osl])
                    o_sb = sp.tile([DP, SEGR_C[c]], F32, tag="o", bufs=2,
                                   padded_shape=[DP, 256])
                    nc.vector.tensor_add(o_sb[:], o_ps[:], nfo_sb[:])
                    nc.gpsimd.dma_start(out_d[:, osl], o_sb[:])

    nc.compile()
    return nc


_CACHE = {}


def _get_nc(key, tiles_w):
    if key not in _CACHE:
        nc = bacc.Bacc("TRN2", target_bir_lowering=False, debug=False,
                       num_devices=NCORE)
        _CACHE[key] = _build(nc, tiles_w)
    return _CACHE[key]


def _prepare(inputs):
    xyz = np.asarray(inputs["xyz"], np.float32)
    xfeat = np.asarray(inputs["xyz_features"], np.float32)
    node = np.asarray(inputs["node"], np.float32)
    nfeat = np.asarray(inputs["node_features"], np.float32)
    idx = np.asarray(inputs["idx"])
    g = {k: np.asarray(inputs[k], np.float32) for k in (
        "fc1_0_w", "fc1_0_b", "fc1_1_w", "fc1_1_b", "fc2_w", "fc2_b",
        "fd_w1", "fd_b1", "fd_w2", "fd_b2", "fg_w1", "fg_b1", "fg_w2", "fg_b2",
        "wq_w", "wk_w", "wv_w")}

    # ---- per-core sort/pad metadata ----
    # Two-level load balancing, both pure host-side indexing:
    # 1. bin-pack segments into windows (greedy, heaviest first into the
    #    lightest window) so global window counts hug 4096 -- windows at
    #    <=4096 need only 8 tiles/core instead of 9;
    # 2. deal window-sorted points round-robin across the 4 cores so
    #    per-core counts are global/4 +- 1.
    wseg = []                  # [B][M] segment -> window
    sval = []                  # [B][M] segment -> slot within window
    segrow = []                # [B][M] row (win*128+slot) -> segment
    for b in range(B):
        scnt = np.bincount(idx[b].astype(np.int64), minlength=M)
        order = np.argsort(-scnt, kind="stable")
        wsum = np.zeros(NWIN, np.int64)
        wfill = np.zeros(NWIN, np.int64)
        ws = np.zeros(M, np.int64)
        sv = np.zeros(M, np.int64)
        for s in order:
            cand = np.where(wfill < 128)[0]
            w = cand[np.argmin(wsum[cand])]
            ws[s] = w
            sv[s] = wfill[w]
            wfill[w] += 1
            wsum[w] += scnt[s]
        # order windows by weight so light windows (8 tiles) align
        # across batches at the same window indices
        worder = np.argsort(wsum, kind="stable")
        wrank = np.empty(NWIN, np.int64)
        wrank[worder] = np.arange(NWIN)
        ws = wrank[ws]
        sr = np.zeros(M, np.int64)
        sr[ws * 128 + sv] = np.arange(M)
        wseg.append(ws)
        sval.append(sv)
        segrow.append(sr)

    cores = []
    counts = np.zeros((NCORE, NWIN), np.int64)
    gperm = [np.argsort(wseg[b][idx[b].astype(np.int64)] * M
                        + idx[b].astype(np.int64), kind="stable")
             for b in range(B)]
    for c in range(NCORE):
        b, r = divmod(c, GROUP)
        pidx = gperm[b][r::GROUP]          # original point ids, sorted by win
        sidx = idx[b].astype(np.int64)[pidx]
        win = wseg[b][sidx]
        counts[c] = np.bincount(win, minlength=NWIN)
        cores.append((b, pidx, sidx, win))

    tiles_w = np.maximum(1, -(-counts.max(axis=0) // 128))
    pad4 = (-int(tiles_w.sum())) % 4
    tiles_w[-1] += pad4
    T = int(tiles_w.sum())

    def bf(x):
        return np.ascontiguousarray(x).astype(NPBF16)

    # ---- shared weight-derived inputs ----
    c_s = g["fd_b2"] - g["wk_w"] @ g["fc1_0_b"]          # folded into t's bias
    c_w = g["wv_w"] @ g["fc1_0_b"] + g["fd_b2"]          # folded into out bias
    wpk = np.zeros((128, 1024), np.float32)
    wpk[:, WC_FD2:WC_FD2 + 128] = g["fd_w2"].T
    wpk[:, WC_FG1:WC_FG1 + 128] = g["fg_w1"].T
    wpk[:, WC_RHSA:WC_RHSA + 128] = g["fg_w2"].T
    wpk[:, WC_ID:WC_ID + 128] = np.eye(DM)
    wpk[0:3, WC_K3:WC_K3 + 128] = (-(g["wk_w"] @ g["fc1_0_w"])).T
    wpk[0:3, WC_V3:WC_V3 + 128] = (g["wv_w"] @ g["fc1_0_w"]).T
    wpk[0:3, WC_FD1A:WC_FD1A + 128] = g["fd_w1"].T
    wpk[3, WC_FD1A:WC_FD1A + 128] = g["fd_b1"]
    wpk[:, WC_FC2:WC_FC2 + 3] = g["fc2_w"].T
    # fp8 weight pairs, plane order (xf, pe1): [k3n|fd2fg1], [v3|fd2T]
    wf8 = np.zeros((128, 512), np.float32)
    wf8[0:3, 0:128] = (g["fg_w1"] @ -(g["wk_w"] @ g["fc1_0_w"])).T
    wf8[:, 128:256] = (g["fg_w1"] @ g["fd_w2"]).T
    wf8[0:3, 256:384] = (g["wv_w"] @ g["fc1_0_w"]).T
    wf8[:, 384:512] = g["fd_w2"].T
    shared = {
        "wpk": bf(wpk),
        "wf8": wf8.astype(NPF8),
        "bt": np.ascontiguousarray(g["fg_b1"][:, None], np.float32),
    }

    # M-scale q table per batch, pre-multiplied by fg1 and including
    # the k/pe bias constant c_s: fqg = fg1 @ (q + c_s)  [128, M]
    q_full = [g["fg_w1"] @ (
        g["wq_w"] @ (g["fc1_1_w"] @ nfeat[b] + g["fc1_1_b"][:, None])
        + c_s[:, None]) for b in range(B)]
    nfo_full = [nfeat[b] + (g["fc2_b"] + g["fc2_w"] @ c_w)[:, None]
                for b in range(B)]

    in_maps = []
    for c in range(NCORE):
        b, pidx, sidx, win = cores[c]
        r = c % GROUP
        cnt = counts[c]
        npts = len(pidx)
        wstart = np.concatenate([[0], np.cumsum(cnt)[:-1]])
        O = 128 * np.concatenate([[0], np.cumsum(tiles_w)[:-1]])
        dest = (O[win] + (np.arange(npts) - wstart[win])).astype(np.int64)

        xf_s = xfeat[b].T[pidx]                           # [npts, 3]
        d_s = xyz[b].T[pidx] - node[b].T[sidx]            # [npts, 3]
        # bulk table: d4a [4, T*128] bf16 (row 3 = ones)
        d4 = np.zeros((4, T * 128), np.float32)
        d4[3] = 1.0
        d4[0:3, dest] = d_s.T
        m = dict(shared)
        m["d4a"] = bf(d4)

        # stream byte-pack per macro [128, 2048] u8:
        # 0:1024 qg bf16 | 1024:1536 oh fp8 | 1536:2048 xf fp8 (rows 0:3)
        qg = np.zeros((128, T * 128), np.float32)
        qg[:, dest] = q_full[b][:, sidx]
        qg_m = bf(qg.reshape(128, T // 4, 512).transpose(1, 0, 2))
        slc = np.full(T * 128, -1, np.int64)
        slc[dest] = sval[b][sidx]
        oh3 = (slc.reshape(T, 128)[:, :, None]
               == np.arange(128)[None, None, :])          # [T, pt, seg]
        oh4 = oh3.reshape(T // 4, 4, 128, 128)
        oh_m = np.ascontiguousarray(
            oh4.transpose(0, 2, 1, 3).reshape(T // 4, 128, 512)).astype(NPF8)
        xf8 = np.zeros((3, T * 128), np.float32)
        xf8[:, dest] = xf_s.T
        xf_m = np.zeros((T // 4, 128, 512), NPF8)
        xf_m[:, 0:3, :] = (xf8.reshape(3, T // 4, 512).transpose(1, 0, 2)
                           .astype(NPF8))
        m["qgoh"] = np.concatenate(
            [qg_m.view(np.uint8), oh_m.view(np.uint8), xf_m.view(np.uint8)],
            axis=2)

        nfo = np.concatenate(
            [nfo_full[b][:, segrow[b][128 * CW[ch] + r * SEGR_C[ch]:
                                      128 * CW[ch] + (r + 1) * SEGR_C[ch]]]
             for ch in range(NCHUNK)], axis=1)
        m["nfo"] = np.ascontiguousarray(nfo, np.float32)
        in_maps.append(m)

    return tiles_w, in_maps, segrow


def _assemble(results, segrow):
    out = np.zeros((B, DP, M), np.float32)
    for c in range(NCORE):
        b, r = divmod(c, GROUP)
        o = results[c]["out"]                             # [3, SEGR_TOT]
        for ch in range(NCHUNK):
            s0 = 128 * CW[ch] + r * SEGR_C[ch]
            out[b][:, segrow[b][s0:s0 + SEGR_C[ch]]] = \
                o[:, OFF_C[ch]:OFF_C[ch] + SEGR_C[ch]]
    return out


def kernel(**inputs):
    tiles_w, in_maps, segrow = _prepare(inputs)
    T = int(tiles_w.sum())
    nc = _get_nc((T, tuple(int(x) for x in tiles_w)), tiles_w)

    import os
    trace = bool(os.environ.get("KERNEL_TRACE"))
    res = run_bass_kernel_spmd(nc, in_maps, list(range(NCORE)), trace=trace,
                               tmpdir=os.environ.get("KERNEL_TRACE_DIR") or None)
    if res.exec_time_ns is not None:
        print(f"HW exec time: {res.exec_time_ns} ns")
    return _assemble(res.results, segrow)


# revision 18
# speedup vs baseline: 1.0681x; 1.0681x over previous
"""GroupPointTransformer Trainium2 kernel (8 NeuronCores).

Strategy:
  - batch b (2) x 4-way shard of the N=131072 points -> 8 cores.
  - Host: per (b, shard) sort points by segment id, pad each 128-segment
    window to whole 128-point tiles (common schedule across cores so the
    SPMD program is input-value independent in shape). Host also folds
    weight products (fg1@fd2, fg1@k3n, fg1-premultiplied q table fqg =
    fg1@(q+c_s) gathered per point) and byte-packs the streams; all
    N-scale GEMMs/nonlinearities stay on device.
  - Device per 512-point macro (emission skewed 4 macros: front half
    [DMA, pe1, t] runs ahead of back half [a, w, ev, scatter]):
      pe1 = relu(fd1a @ [d;1])           (bias via K=4 ones-row; Act
                                          writes fp8 into pe1 tile)
      t   = relu(fd2fg1@pe1 + k3fg1@xf + ident@fqg + fg_b1)
            -- the s stage folded through fg1; xf enters as a K=3
            matmul straight from the bulk xf table; relu on DVE
      a,w = point-major data-stationary matmuls (w as two unpaired MMs:
            pe1-slice@fd2T fp8 + xf-slice@v3); e = exp(a/sqrt(128))
      segment sums of [e, e*(v+pe)] via fp8 one-hot scatter matmuls;
      adjacent same-window tiles pair into fp8 DoubleRow matmuls (2x).
    Bias/softmax folds: fg_b2 dropped (per-feature softmax invariant),
    max-subtraction dropped (logits O(0.01), shift-invariant), w bias
    cw=wv@b0+fd_b2 folded into the output bias since sum(attn)=1.
  - DMA: d4/xf ship once as compact bulk tables (the old per-macro
    xfd8 stream was ~97% zero padding); per-macro only the qgoh stream
    (bf16 fqg + fp8 one-hot) via Sync HWDGE. cc_in/tail DMAs ride Pool
    SWDGE so they never contend with the stream ring.
  - ReduceScatter (bf16) per 4-core group, pipelined chunks; the whole
    tail (res = numer/denom, out = fc2 @ res + nfo) is deferred past
    the main loop via tile_wait_until so RS-completion waits never
    head-of-line-block a stream queue mid-kernel.
"""

import math

import ml_dtypes
import numpy as np

import concourse.bacc as bacc
import concourse.bass as bass
import concourse.mybir as mybir
import concourse.tile as tile
from concourse.bass_utils import run_bass_kernel_spmd

B, N, M, DP, DM = 2, 131072, 4096, 3, 128
NCORE = 8
GROUP = 4                    # cores per batch
NS = N // GROUP              # points per core = 32768
NWIN = M // 128              # 32 windows of 128 segments
BF16 = mybir.dt.bfloat16
F32 = mybir.dt.float32
F8 = mybir.dt.float8e4
F16 = mybir.dt.float16
U8 = mybir.dt.uint8
NPBF16 = ml_dtypes.bfloat16
NPF8 = ml_dtypes.float8_e4m3
ISQ = 1.0 / math.sqrt(DM)
RG = [[0, 1, 2, 3], [4, 5, 6, 7]]
# reduce-scatter pipeline chunks (windows per chunk)
W_CH = [4, 4, 4, 4, 4, 4, 4, 4]
NCHUNK = len(W_CH)
CW = [0]
for _w in W_CH:
    CW.append(CW[-1] + _w)
SEGR_C = [32 * w for w in W_CH]      # per-core rows per chunk
OFF_C = [0]
for _s in SEGR_C:
    OFF_C.append(OFF_C[-1] + _s)
SEGR_TOT = OFF_C[-1]                 # = M // GROUP = 1024

# weight-pack column offsets in wpk [128, 1024] bf16
WC_FD2, WC_FG1, WC_RHSA, WC_ID = 0, 128, 256, 384
WC_K3, WC_V3, WC_FD1A, WC_FC2 = 512, 640, 768, 896


def _build(nc, tiles_w, no_cc=False):
    """Emit the SPMD program. tiles_w[w] = # of 128-point tiles in window w
    (common across all cores)."""
    T = int(sum(tiles_w))
    assert T % 4 == 0
    nmacro = T // 4
    win_of = np.repeat(np.arange(NWIN), tiles_w)
    first_t = np.zeros(NWIN, np.int64)
    last_t = np.zeros(NWIN, np.int64)
    o = 0
    for w in range(NWIN):
        first_t[w] = o
        o += tiles_w[w]
        last_t[w] = o - 1

    # ---- I/O ----
    # bulk point table: d4a = [d;1] bf16 (one upfront DMA, SBUF-resident)
    d4a_d = nc.dram_tensor("d4a", [4, T * 128], BF16, kind="ExternalInput")
    # byte-packed per-macro stream: 0:1024 qg bf16 | 1024:1536 oh fp8 |
    # 1536:2048 xf fp8 (rows 0:3; rest zeros).  The SBUF tile has a pe1
    # slot at 2048:2560 so (xf, pe1) form adjacent DoubleRow planes.
    qgoh_d = nc.dram_tensor("qgoh", [T // 4, 128, 2048], U8,
                            kind="ExternalInput")
    wpk_d = nc.dram_tensor("wpk", [128, 1024], BF16, kind="ExternalInput")
    wf8_d = nc.dram_tensor("wf8", [128, 512], F8, kind="ExternalInput")
    bt_d = nc.dram_tensor("bt", [DM, 1], F32, kind="ExternalInput")
    nfo_d = nc.dram_tensor("nfo", [DP, SEGR_TOT], F32, kind="ExternalInput")
    out_d = nc.dram_tensor("out", [DP, SEGR_TOT], F32, kind="ExternalOutput")

    cc_in = nc.dram_tensor("cc_in", [M, 256], F16)
    cc_out = [nc.dram_tensor(f"cc_out{c}", [SEGR_C[c], 256], F16)
              for c in range(NCHUNK)]

    AF = mybir.ActivationFunctionType
    AL = mybir.AluOpType

    with tile.TileContext(nc) as tc:
        with (
            tc.tile_pool(name="cpool", bufs=1) as cp,
            tc.tile_pool(name="spool", bufs=8) as sp,
            tc.tile_pool(name="psum", bufs=1, space="PSUM") as pp,
        ):
            # ---- constants ----
            wpk = cp.tile([128, 1024], BF16, tag="wpk", name="wpk")
            nc.sync.dma_start(wpk[:], wpk_d[:])

            fd2T = wpk[:, WC_FD2:WC_FD2 + 128]
            rhsa = wpk[:, WC_RHSA:WC_RHSA + 128]
            ident = wpk[:, WC_ID:WC_ID + 128]
            fd1aT = wpk[0:4, WC_FD1A:WC_FD1A + 128]
            fc2T = wpk[:, WC_FC2:WC_FC2 + 3]
            # fp8 weights: [fd2fg1T | k3fg1T | fd2T | v3]
            # fp8 weight pairs, plane order (xf, pe1): [k3n|fd2fg1], [v3|fd2T]
            wf8 = cp.tile([128, 512], F8, tag="wf8", name="wf8")
            bt = cp.tile([DM, 1], F32, tag="bt", name="bt")
            kfd8 = wf8[:, 0:256].rearrange("p (a f) -> p a f", a=2)
            vfd8 = wf8[:, 256:512].rearrange("p (a f) -> p a f", a=2)

            d4a = cp.tile([4, T * 128], BF16, tag="d4a", name="d4a")

            # ---- main loop over 512-point macros ----
            # software-pipelined emission: front half (DMA, pe1, t) of macro
            # mi is emitted before the back half (a, w, ev, scatter) of
            # macro mi-1, so each engine's program order matches data-ready
            # order (Act/DVE have shallow lookahead queues).
            live = {}
            closed = np.zeros(NWIN, bool)
            fr = {}
            fr_sc = {}

            pf = {}

            def prefetch(mi):
                qc = sp.tile([128, 2560], U8, tag="qgoh", name="qc")
                nc.sync.dma_start(qc[:, 0:2048], qgoh_d[mi][:])
                pf[mi] = qc

            def front(mi):
                qc = pf.pop(mi)
                fqg = qc[:, 0:1024].bitcast(BF16)
                sl = slice(mi * 512, (mi + 1) * 512)
                pe1_ps = pp.tile([128, 512], F32, tag="pe1", bufs=1)
                nc.tensor.matmul(pe1_ps[:], fd1aT, d4a[:, sl],
                                 start=True, stop=True)
                nc.scalar.activation(qc[:, 2048:2560].bitcast(F8), pe1_ps[:],
                                     AF.Relu)

                # t = (fg1 fd2)@pe1 + (fg1 k3n)@xf + fg1@(q+c_s)[idx]:
                # one fp8 DoubleRow over (xf, pe1) + one ident matmul for fqg
                t_ps = pp.tile([128, 512], F32, tag="t", bufs=2)
                comb2 = qc[:, 1536:2560].bitcast(F8).rearrange(
                    "p (a f) -> p a f", a=2)
                nc.tensor.matmul(t_ps[:], kfd8, comb2, start=True, stop=False,
                                 perf_mode=mybir.MatmulPerfMode.DoubleRow)
                nc.tensor.matmul(t_ps[:], ident, fqg, start=False, stop=True)
                t_sb = sp.tile([128, 512], BF16, tag="t")
                # relu lives on DVE; Act keeps pe1-relu + exp + sc copies
                nc.vector.tensor_scalar(t_sb[:], t_ps[:], bt[:], 0.0,
                                        AL.add, AL.max)
                fr[mi] = (qc, t_sb)

            def back(mi):
                t0 = 4 * mi
                qc, t_sb = fr.pop(mi)
                oh4 = qc[:, 1024:1536].bitcast(F8).rearrange(
                    "p (a f) -> p a f", a=4)
                comb2 = qc[:, 1536:2560].bitcast(F8).rearrange(
                    "p (a f) -> p a f", a=2)

                # point-major: w = (v + pe)^T, a = (fg2 t)^T
                w_ps = pp.tile([128, 4, 128], F32, tag="w", bufs=2, name="w_ps")
                a_ps = pp.tile([128, 4, 128], F32, tag="a", bufs=1, name="a_ps")
                for k in range(4):
                    sl = slice(k * 128, (k + 1) * 128)
                    nc.tensor.matmul(a_ps[:, k, :], t_sb[:, sl], rhsa,
                                     start=True, stop=True)
                for k in range(4):
                    nc.tensor.matmul(w_ps[:, k, :],
                                     comb2[:, :, k * 128:(k + 1) * 128], vfd8,
                                     start=True, stop=True,
                                     perf_mode=mybir.MatmulPerfMode.DoubleRow)

                # e and e*w interleaved per subtile: [pts, (k, e|ew)], fp8
                ev_sb = sp.tile([128, 4, 256], F8, tag="ev")
                nc.scalar.activation(ev_sb[:, :, 0:128], a_ps[:], AF.Exp,
                                     scale=ISQ)
                nc.vector.tensor_mul(ev_sb[:, :, 128:256], ev_sb[:, :, 0:128],
                                     w_ps[:])

                # scatter into per-window PSUM accumulators; adjacent tiles of
                # the same window pair up into one fp8 DoubleRow matmul (2x)
                k = 0
                while k < 4:
                    t = t0 + k
                    w = int(win_of[t])
                    if w // 2 not in live:
                        live[w // 2] = pp.tile([128, 512], F32, tag="sc",
                                               bufs=2, name=f"sc{w // 2}")
                    paired = k < 3 and int(win_of[t + 1]) == w
                    st = t == first_t[w]
                    reg = live[w // 2][:, (w % 2) * 256:(w % 2) * 256 + 256]
                    if paired:
                        fin = t + 1 == last_t[w]
                        nc.tensor.matmul(reg, oh4[:, k:k + 2, :],
                                         ev_sb[:, k:k + 2, :], start=st, stop=fin,
                                         perf_mode=mybir.MatmulPerfMode.DoubleRow)
                        k += 2
                    else:
                        fin = t == last_t[w]
                        nc.tensor.matmul(reg, oh4[:, k, :],
                                         ev_sb[:, k, :], start=st, stop=fin)
                        k += 1
                    if fin:
                        closed[w] = True
                        # drain each window half as it closes so the shared
                        # PSUM bank frees right after the second half-copy
                        if w % 2 == 0:
                            sc2 = sp.tile([128, 2, 256], F16, tag="scsb",
                                          name="sc2")
                            fr_sc[w // 2] = sc2
                            nc.scalar.copy(sc2[:, 0, :],
                                           live[w // 2][:, 0:256])
                        else:
                            sc2 = fr_sc.pop(w // 2)
                            if (w // 2) % 2 == 0:
                                nc.scalar.copy(sc2[:, 1, :],
                                               live[w // 2][:, 256:512])
                            else:
                                nc.vector.tensor_copy(sc2[:, 1, :],
                                                      live[w // 2][:, 256:512])
                            nc.sync.dma_start(
                                cc_in[(w - 1) * 128:(w + 1) * 128, :]
                                .rearrange("(a p) f -> p a f", p=128), sc2[:])
                            del live[w // 2]
                        for c in range(NCHUNK):
                            if (not no_cc and CW[c] <= w < CW[c + 1]
                                    and closed[CW[c]:CW[c + 1]].all()):
                                nc.gpsimd.collective_compute(
                                    "ReduceScatter", AL.add, replica_groups=RG,
                                    ins=[cc_in[CW[c] * 128:CW[c + 1] * 128, :]],
                                    outs=[cc_out[c][:]])

            # d4a gates the pe1 matmuls; land the first macros' slice fast,
            # then trickle the bulk in chunks so no long-descriptor DMA
            # hogs the SDMA engines during fill
            nc.sync.dma_start(d4a[:, 0:4096], d4a_d[:, 0:4096])
            prefetch(0)
            nc.sync.dma_start(wf8[:], wf8_d[:])
            nc.scalar.dma_start(bt[:], bt_d[:])
            prefetch(1)
            nbulk = (T * 128 - 4096 + 3) // 4
            SKEW = 4
            for mi in range(nmacro + SKEW):
                if 2 <= mi < 6:
                    o = 4096 + (mi - 2) * nbulk
                    e = min(T * 128, o + nbulk)
                    nc.scalar.dma_start(d4a[:, o:e], d4a_d[:, o:e])
                if mi + 2 < nmacro:
                    prefetch(mi + 2)
                if mi < nmacro:
                    front(mi)
                if mi >= SKEW:
                    back(mi - SKEW)

            # ---- tail: res = numer/denom; out = fc2 @ res + nfo ----
            # deferred past the whole main loop (tile_wait_until) so the
            # RS-completion waits sit at the end of every engine queue and
            # never head-of-line-block the stream mid-kernel.
            with tc.tile_wait_until(10.0):
                for c in range(NCHUNK):
                    NA = SEGR_C[c] // 128
                    osl = slice(OFF_C[c], OFF_C[c + 1])
                    tt = sp.tile([128, NA, 256], F16, tag="tt", bufs=2,
                                 padded_shape=[128, 2, 256])
                    nc.sync.dma_start(
                        tt[:], cc_out[c].rearrange("(a p) f -> p a f", p=128))
                    dmx = sp.tile([128, NA, 128], F32, tag="dmx", bufs=2,
                                  padded_shape=[128, 2, 128])
                    nc.vector.tensor_scalar_max(dmx[:], tt[:, :, 0:128], 1e-30)
                    rec = sp.tile([128, NA, 128], F32, tag="rec", bufs=2,
                                  padded_shape=[128, 2, 128])
                    nc.vector.reciprocal_approx_fast(rec[:], dmx[:])
                    res = sp.tile([128, NA, 128], BF16, tag="res", bufs=2,
                                  padded_shape=[128, 2, 128])
                    nc.vector.tensor_mul(res[:], tt[:, :, 128:256], rec[:])
                    rT_ps = pp.tile([128, SEGR_C[c]], BF16, tag="pe1", bufs=1,
                                    name="rT_ps", padded_shape=[128, 256])
                    for a in range(NA):
                        nc.tensor.transpose(rT_ps[:, a * 128:(a + 1) * 128],
                                            res[:, a, :], ident)
                    rT_sb = sp.tile([128, SEGR_C[c]], BF16, tag="rT", bufs=2,
                                    padded_shape=[128, 256])
                    nc.scalar.copy(rT_sb[:], rT_ps[:])
                    o_ps = pp.tile([DP, SEGR_C[c]], F32, tag="a", bufs=1,
                                   name="o_ps", padded_shape=[DP, 256])
                    nc.tensor.matmul(o_ps[:], fc2T, rT_sb[:], start=True,
                                     stop=True)
                    nfo_sb = sp.tile([DP, SEGR_C[c]], F32, tag="nfo", bufs=2,
                                     padded_shape=[DP, 256])
                    nc.sync.dma_start(nfo_sb[:], nfo_d[:, osl])
                    o_sb = sp.tile([DP, SEGR_C[c]], F32, tag="o", bufs=2,
                                   padded_shape=[DP, 256])
                    nc.vector.tensor_add(o_sb[:], o_ps[:], nfo_sb[:])
                    nc.sync.dma_start(out_d[:, osl], o_sb[:])

    nc.compile()
    return nc


_CACHE = {}


def _get_nc(key, tiles_w):
    if key not in _CACHE:
        nc = bacc.Bacc("TRN2", target_bir_lowering=False, debug=False,
                       num_devices=NCORE)
        _CACHE[key] = _build(nc, tiles_w)
    return _CACHE[key]


def _prepare(inputs):
    xyz = np.asarray(inputs["xyz"], np.float32)
    xfeat = np.asarray(inputs["xyz_features"], np.float32)
    node = np.asarray(inputs["node"], np.float32)
    nfeat = np.asarray(inputs["node_features"], np.float32)
    idx = np.asarray(inputs["idx"])
    g = {k: np.asarray(inputs[k], np.float32) for k in (
        "fc1_0_w", "fc1_0_b", "fc1_1_w", "fc1_1_b", "fc2_w", "fc2_b",
        "fd_w1", "fd_b1", "fd_w2", "fd_b2", "fg_w1", "fg_b1", "fg_w2", "fg_b2",
        "wq_w", "wk_w", "wv_w")}

    # ---- per-core sort/pad metadata ----
    # Two-level load balancing, both pure host-side indexing:
    # 1. bin-pack segments into windows (greedy, heaviest first into the
    #    lightest window) so global window counts hug 4096 -- windows at
    #    <=4096 need only 8 tiles/core instead of 9;
    # 2. deal window-sorted points round-robin across the 4 cores so
    #    per-core counts are global/4 +- 1.
    wseg = []                  # [B][M] segment -> window
    sval = []                  # [B][M] segment -> slot within window
    segrow = []                # [B][M] row (win*128+slot) -> segment
    for b in range(B):
        scnt = np.bincount(idx[b].astype(np.int64), minlength=M)
        order = np.argsort(-scnt, kind="stable")
        wsum = np.zeros(NWIN, np.int64)
        wfill = np.zeros(NWIN, np.int64)
        ws = np.zeros(M, np.int64)
        sv = np.zeros(M, np.int64)
        for s in order:
            cand = np.where(wfill < 128)[0]
            w = cand[np.argmin(wsum[cand])]
            ws[s] = w
            sv[s] = wfill[w]
            wfill[w] += 1
            wsum[w] += scnt[s]
        # order windows by weight so light windows (8 tiles) align
        # across batches at the same window indices
        worder = np.argsort(wsum, kind="stable")
        wrank = np.empty(NWIN, np.int64)
        wrank[worder] = np.arange(NWIN)
        ws = wrank[ws]
        sr = np.zeros(M, np.int64)
        sr[ws * 128 + sv] = np.arange(M)
        wseg.append(ws)
        sval.append(sv)
        segrow.append(sr)

    cores = []
    counts = np.zeros((NCORE, NWIN), np.int64)
    gperm = [np.argsort(wseg[b][idx[b].astype(np.int64)] * M
                        + idx[b].astype(np.int64), kind="stable")
             for b in range(B)]
    for c in range(NCORE):
        b, r = divmod(c, GROUP)
        pidx = gperm[b][r::GROUP]          # original point ids, sorted by win
        sidx = idx[b].astype(np.int64)[pidx]
        win = wseg[b][sidx]
        counts[c] = np.bincount(win, minlength=NWIN)
        cores.append((b, pidx, sidx, win))

    tiles_w = np.maximum(1, -(-counts.max(axis=0) // 128))
    pad4 = (-int(tiles_w.sum())) % 4
    tiles_w[-1] += pad4
    T = int(tiles_w.sum())

    def bf(x):
        return np.ascontiguousarray(x).astype(NPBF16)

    # ---- shared weight-derived inputs ----
    c_s = g["fd_b2"] - g["wk_w"] @ g["fc1_0_b"]          # folded into t's bias
    c_w = g["wv_w"] @ g["fc1_0_b"] + g["fd_b2"]          # folded into out bias
    wpk = np.zeros((128, 1024), np.float32)
    wpk[:, WC_FD2:WC_FD2 + 128] = g["fd_w2"].T
    wpk[:, WC_FG1:WC_FG1 + 128] = g["fg_w1"].T
    wpk[:, WC_RHSA:WC_RHSA + 128] = g["fg_w2"].T
    wpk[:, WC_ID:WC_ID + 128] = np.eye(DM)
    wpk[0:3, WC_K3:WC_K3 + 128] = (-(g["wk_w"] @ g["fc1_0_w"])).T
    wpk[0:3, WC_V3:WC_V3 + 128] = (g["wv_w"] @ g["fc1_0_w"]).T
    wpk[0:3, WC_FD1A:WC_FD1A + 128] = g["fd_w1"].T
    wpk[3, WC_FD1A:WC_FD1A + 128] = g["fd_b1"]
    wpk[:, WC_FC2:WC_FC2 + 3] = g["fc2_w"].T
    # fp8 weight pairs, plane order (xf, pe1): [k3n|fd2fg1], [v3|fd2T]
    wf8 = np.zeros((128, 512), np.float32)
    wf8[0:3, 0:128] = (g["fg_w1"] @ -(g["wk_w"] @ g["fc1_0_w"])).T
    wf8[:, 128:256] = (g["fg_w1"] @ g["fd_w2"]).T
    wf8[0:3, 256:384] = (g["wv_w"] @ g["fc1_0_w"]).T
    wf8[:, 384:512] = g["fd_w2"].T
    shared = {
        "wpk": bf(wpk),
        "wf8": wf8.astype(NPF8),
        "bt": np.ascontiguousarray(g["fg_b1"][:, None], np.float32),
    }

    # M-scale q table per batch, pre-multiplied by fg1 and including
    # the k/pe bias constant c_s: fqg = fg1 @ (q + c_s)  [128, M]
    q_full = [g["fg_w1"] @ (
        g["wq_w"] @ (g["fc1_1_w"] @ nfeat[b] + g["fc1_1_b"][:, None])
        + c_s[:, None]) for b in range(B)]
    nfo_full = [nfeat[b] + (g["fc2_b"] + g["fc2_w"] @ c_w)[:, None]
                for b in range(B)]

    in_maps = []
    for c in range(NCORE):
        b, pidx, sidx, win = cores[c]
        r = c % GROUP
        cnt = counts[c]
        npts = len(pidx)
        wstart = np.concatenate([[0], np.cumsum(cnt)[:-1]])
        O = 128 * np.concatenate([[0], np.cumsum(tiles_w)[:-1]])
        dest = (O[win] + (np.arange(npts) - wstart[win])).astype(np.int64)

        xf_s = xfeat[b].T[pidx]                           # [npts, 3]
        d_s = xyz[b].T[pidx] - node[b].T[sidx]            # [npts, 3]
        # bulk table: d4a [4, T*128] bf16 (row 3 = ones)
        d4 = np.zeros((4, T * 128), np.float32)
        d4[3] = 1.0
        d4[0:3, dest] = d_s.T
        m = dict(shared)
        m["d4a"] = bf(d4)

        # stream byte-pack per macro [128, 2048] u8:
        # 0:1024 qg bf16 | 1024:1536 oh fp8 | 1536:2048 xf fp8 (rows 0:3)
        qg = np.zeros((128, T * 128), np.float32)
        qg[:, dest] = q_full[b][:, sidx]
        qg_m = bf(qg.reshape(128, T // 4, 512).transpose(1, 0, 2))
        slc = np.full(T * 128, -1, np.int64)
        slc[dest] = sval[b][sidx]
        oh3 = (slc.reshape(T, 128)[:, :, None]
               == np.arange(128)[None, None, :])          # [T, pt, seg]
        oh4 = oh3.reshape(T // 4, 4, 128, 128)
        oh_m = np.ascontiguousarray(
            oh4.transpose(0, 2, 1, 3).reshape(T // 4, 128, 512)).astype(NPF8)
        xf8 = np.zeros((3, T * 128), np.float32)
        xf8[:, dest] = xf_s.T
        xf_m = np.zeros((T // 4, 128, 512), NPF8)
        xf_m[:, 0:3, :] = (xf8.reshape(3, T // 4, 512).transpose(1, 0, 2)
                           .astype(NPF8))
        m["qgoh"] = np.concatenate(
            [qg_m.view(np.uint8), oh_m.view(np.uint8), xf_m.view(np.uint8)],
            axis=2)

        nfo = np.concatenate(
            [nfo_full[b][:, segrow[b][128 * CW[ch] + r * SEGR_C[ch]:
                                      128 * CW[ch] + (r + 1) * SEGR_C[ch]]]
             for ch in range(NCHUNK)], axis=1)
        m["nfo"] = np.ascontiguousarray(nfo, np.float32)
        in_maps.append(m)

    return tiles_w, in_maps, segrow


def _assemble(results, segrow):
    out = np.zeros((B, DP, M), np.float32)
    for c in range(NCORE):
        b, r = divmod(c, GROUP)
        o = results[c]["out"]                             # [3, SEGR_TOT]
        for ch in range(NCHUNK):
            s0 = 128 * CW[ch] + r * SEGR_C[ch]
            out[b][:, segrow[b][s0:s0 + SEGR_C[ch]]] = \
                o[:, OFF_C[ch]:OFF_C[ch] + SEGR_C[ch]]
    return out


def kernel(**inputs):
    tiles_w, in_maps, segrow = _prepare(inputs)
    T = int(tiles_w.sum())
    nc = _get_nc((T, tuple(int(x) for x in tiles_w)), tiles_w)

    import os
    trace = bool(os.environ.get("KERNEL_TRACE"))
    res = run_bass_kernel_spmd(nc, in_maps, list(range(NCORE)), trace=trace,
                               tmpdir=os.environ.get("KERNEL_TRACE_DIR") or None)
    if res.exec_time_ns is not None:
        print(f"HW exec time: {res.exec_time_ns} ns")
    return _assemble(res.results, segrow)
